# revision 2
# baseline (speedup 1.0000x reference)
"""BasicGAT Trainium2 kernel — 8-core SPMD, transfer-optimized.

The wall-clock metric is dominated by host->device input transfer over the
axon tunnel plus per-call jit/compile overhead, so this version:

- Enables the JAX persistent compilation cache (warm calls skip BIR
  re-verification / DVE table regen: ~500ms -> ~110ms fixed).
- Ships x as fp8_e4m3 (rel err contribution ~1e-3, budget 2e-2).
- Packs ALL per-core inputs into one int16 blob and ALL shared inputs into a
  second int16 blob that is sharded 8 ways and AllGather'ed on device
  (weights transfer once instead of 8x).
- Replaces the 25 MB host-built im2col table (xim) + redundant D1 conv with
  an on-device computation of the layer-1 att_dst row: each core computes
  ad = (W0 @ a_d) . conv_out for its own slice, a tiny AllGather shares all
  16384 values, and a 64-f32-wide dma_gather + one-hot column select
  extracts the D1 window layout.

Everything else follows the proven baseline structure:
- Temporal conv (+residual+relu) on a 2048-node slice per core.
- Dead-code elimination: output depends only on nodes {b*4096+0, b*4096+1};
  backward closure gives per-layer dst/edge sets.
- Layer 1 src-partitioned with per-window partial aggregation + AllReduce.
- Layers 2/3 + final conv + layernorm computed redundantly on every core.
"""
import sys, os
sys.path.insert(0, "/opt/trn_rl_repo")
import numpy as np
import ml_dtypes

BF = ml_dtypes.bfloat16
FP8 = ml_dtypes.float8_e4m3

B, N, F, H, E, L = 4, 4096, 256, 256, 262144, 3
NTOT = B * N
NCORES = 8
SLICE = NTOT // NCORES
EPS = 1e-5
NEG = 0.2


def _enable_jax_cache():
    import jax
    try:
        jax.config.update("jax_compilation_cache_dir",
                          os.environ.get("KJAXCACHE", "/tmp/jaxcache"))
        jax.config.update("jax_persistent_cache_min_compile_time_secs", 0.0)
        jax.config.update("jax_persistent_cache_min_entry_size_bytes", 0)
    except Exception:
        pass


_enable_jax_cache()


# ---------------------------------------------------------------- host prep
def _closure(src, dst):
    D3 = np.array(sorted(b * N + j for b in range(B) for j in (0, 1)), np.int64)

    def back(D):
        m = np.zeros(NTOT, bool)
        m[D] = True
        sel = np.nonzero(m[dst])[0]
        S = np.unique(np.concatenate([src[sel], D]))
        return sel, S

    e3, S3 = back(D3)
    e2, S2 = back(S3)
    e1, S1 = back(S2)
    return D3, e3, S3, e2, S2, e1


def _wrap16(flat):
    """flat int list (len % 16 == 0) -> [16, len//16] int16 (SWDGE layout,
    un-replicated; device replicates to 128 partitions)."""
    return np.ascontiguousarray(np.asarray(flat, np.int16).reshape(-1, 16).T)


def _layer_sched(src_e, dst_e, Dl, src_pos, by_core):
    """Window-aligned chunk schedule (same as baseline)."""
    pos_d = np.full(NTOT, -1, np.int64)
    pos_d[Dl] = np.arange(len(Dl))
    w = pos_d[dst_e] // 128
    slot = pos_d[dst_e] % 128
    nw = (len(Dl) + 127) // 128
    cores = (src_e // SLICE) if by_core else np.zeros(len(src_e), np.int64)
    ncr = NCORES if by_core else 1
    cnt = np.zeros((ncr, nw), np.int64)
    np.add.at(cnt, (cores, w), 1)
    Tw = [max(1, int(np.ceil(cnt[:, j].max() / 128))) for j in range(nw)]
    idxs, slots = [], []
    for c in range(ncr):
        gi, gs = [], []
        for j in range(nw):
            sel = np.nonzero((cores == c) & (w == j))[0]
            n = Tw[j] * 128
            ii = np.zeros(n, np.int64)
            ss = np.full(n, -1.0, np.float32)
            ii[: len(sel)] = src_pos[c][src_e[sel]] if by_core else src_pos[0][src_e[sel]]
            ss[: len(sel)] = slot[sel]
            gi.append(ii)
            gs.append(ss)
        idxs.append(np.concatenate(gi))
        slots.append(np.concatenate(gs))
    return Tw, idxs, slots


class _Blob:
    """Append-only int16 blob with 2-unit (4-byte) alignment per entry."""

    def __init__(self):
        self.parts = []
        self.off = 0
        self.offs = {}

    def add(self, name, arr):
        a = np.ascontiguousarray(arr)
        assert a.dtype.itemsize in (1, 2, 4)
        flat = a.view(np.uint8).ravel()
        assert len(flat) % 2 == 0, name
        u = len(flat) // 2
        if self.off % 2:
            self.parts.append(np.zeros(1, np.int16))
            self.off += 1
        self.offs[name] = (self.off, u)
        self.parts.append(flat.view(np.int16))
        self.off += u

    def finish(self, align):
        pad = (-self.off) % align
        if pad:
            self.parts.append(np.zeros(pad, np.int16))
            self.off += pad
        return np.concatenate(self.parts)


def _prep(x, edge_index, tc_w, tc_b, gat_W, gat_as, gat_ad, gat_b, ln_g, ln_b):
    ei = np.asarray(edge_index)
    src = np.concatenate([ei[0], np.arange(NTOT)]).astype(np.int64)
    dst = np.concatenate([ei[1], np.arange(NTOT)]).astype(np.int64)
    D3, e3, D2, e2, D1, e1 = _closure(src, dst)
    W1N = (len(D1) + 127) // 128
    W2N = (len(D2) + 127) // 128

    xf = np.asarray(x).reshape(NTOT, F).astype(np.float32)

    # L1: src-partitioned; gather table = own conv slice (row = node - c*SLICE)
    sp1 = [np.arange(NTOT) - c * SLICE for c in range(NCORES)]
    T1w, g1i, g1s = _layer_sched(src[e1], dst[e1], D1, sp1, True)
    # L2/L3: shared; gather tables = compact D1 / D2
    pos1 = np.full(NTOT, 0, np.int64)
    pos1[D1] = np.arange(len(D1))
    pos2 = np.full(NTOT, 0, np.int64)
    pos2[D2] = np.arange(len(D2))
    T2w, g2i, g2s = _layer_sched(src[e2], dst[e2], D2, [pos1], False)
    T3w, g3i, g3s = _layer_sched(src[e3], dst[e3], D3, [pos2], False)

    D1N, D2N, D3N = W1N * 128, W2N * 128, 128
    T1t, T2t, T3t = sum(T1w), sum(T2w), sum(T3w)

    # dst-row index lists for ad extraction of layers 2/3
    d2i = np.zeros(D2N, np.int64)
    d2i[: len(D2)] = pos1[D2]
    d3i = np.zeros(D3N, np.int64)
    d3i[: len(D3)] = pos2[D3]

    # layer-1 ad extraction: gather row = node//64 from the AllGather'ed
    # per-node ad table ([NTOT/64, 64] f32), one-hot select col = node%64
    d1r = np.zeros(D1N, np.int64)
    d1r[: len(D1)] = D1 // 64
    d1c = np.full(D1N, -1.0, np.float32)
    d1c[: len(D1)] = D1 % 64
    d1col = np.ascontiguousarray(d1c.reshape(W1N, 128).T).astype(BF)  # [128, W1N]

    # x slices, transposed, with halo + graph-boundary zeros, fp8
    def xt_slice(c):
        lo = c * SLICE
        g0 = lo // N
        out = np.zeros((SLICE + 2, F), np.float32)
        for j in range(SLICE + 2):
            n = lo + j - 1
            if g0 * N <= n < (g0 + 1) * N:
                out[j] = xf[n]
        return np.ascontiguousarray(out.T).astype(FP8)  # [256, 2050]

    tc_w = np.asarray(tc_w, np.float32)
    # t_wconv[p, k, kc, m] = tc_w[m, kc*128+p, k]
    hwconv = np.ascontiguousarray(
        tc_w.transpose(1, 2, 0).reshape(2, 128, 3, H).transpose(1, 2, 0, 3)
    ).astype(BF)
    gat_W = np.asarray(gat_W, np.float32)
    gat_as = np.asarray(gat_as, np.float32)
    gat_ad = np.asarray(gat_ad, np.float32)
    wext = np.stack([
        np.concatenate(
            [gat_W[l], (gat_W[l] @ gat_as[l])[:, None], (gat_W[l] @ gat_ad[l])[:, None]],
            axis=1)
        for l in range(L)
    ])  # [3, 256, 258]
    # t_wext[p, kc, l, m] = wext[l, kc*128+p, m]
    hwext = np.ascontiguousarray(
        wext.reshape(3, 2, 128, 258).transpose(2, 1, 0, 3)).astype(BF)
    # t_wadc[p, kc] = (W0 @ ad)[kc*128+p]
    hwadc = np.ascontiguousarray(
        (gat_W[0] @ gat_ad[0]).reshape(2, 128).T).astype(BF)
    # t_bcol[p, l, kc] = gat_b[l, kc*128+p]
    hbcol = np.ascontiguousarray(
        np.asarray(gat_b, np.float32).reshape(L, 2, 128).transpose(2, 0, 1))
    htcb = np.ascontiguousarray(
        np.asarray(tc_b, np.float32).reshape(2, 128).T)

    sh = _Blob()
    sh.add("wconv", hwconv)
    sh.add("wext", hwext)
    sh.add("wadc", hwadc)
    sh.add("bcol", hbcol)
    sh.add("tcb", htcb)
    sh.add("c128", np.tile(np.arange(128, dtype=np.float32), (128, 1)).astype(BF))
    sh.add("identf", np.eye(128, dtype=np.float32))
    sh.add("lng", np.tile(np.asarray(ln_g, np.float32), (B, 1)))
    sh.add("lnb", np.tile(np.asarray(ln_b, np.float32), (B, 1)))
    sh.add("g2i", _wrap16(g2i[0]))
    sh.add("g2s", np.ascontiguousarray(g2s[0].reshape(T2t, 128).T).astype(BF))
    sh.add("g3i", _wrap16(g3i[0]))
    sh.add("g3s", np.ascontiguousarray(g3s[0].reshape(T3t, 128).T).astype(BF))
    sh.add("d2i", _wrap16(d2i))
    sh.add("d3i", _wrap16(d3i))
    sh.add("d1i", _wrap16(d1r))
    sh.add("d1col", d1col)
    shfull = sh.finish(align=2 * NCORES)
    SW = len(shfull) // NCORES

    meta = dict(T1w=T1w, T2w=T2w, T3w=T3w, W1N=W1N, W2N=W2N,
                D1N=D1N, D2N=D2N, D3N=D3N, T1t=T1t, T2t=T2t, T3t=T3t,
                sh_offs=sh.offs, SW=SW)

    in_maps = []
    for c in range(NCORES):
        pc = _Blob()
        pc.add("xtq", xt_slice(c))
        pc.add("g1i", _wrap16(g1i[c]))
        pc.add("g1s", np.ascontiguousarray(g1s[c].reshape(T1t, 128).T).astype(BF))
        pcb = pc.finish(align=2)
        if c == 0:
            meta["pc_offs"] = pc.offs
            meta["PCW"] = len(pcb)
        in_maps.append({
            "pcb": pcb.reshape(1, -1),
            "shb": shfull[c * SW:(c + 1) * SW].reshape(1, -1),
        })
    return in_maps, meta


# ---------------------------------------------------------------- device program
def _build(meta):
    import concourse.bass as bass
    import concourse.bacc as bacc
    import concourse.tile as tile
    import concourse.mybir as mybir

    F32 = mybir.dt.float32
    BF16 = mybir.dt.bfloat16
    I16 = mybir.dt.int16
    FP8E4 = mybir.dt.float8e4
    Alu = mybir.AluOpType
    Act = mybir.ActivationFunctionType
    adep = bass._add_dep_helper

    T1w, T2w, T3w = meta["T1w"], meta["T2w"], meta["T3w"]
    W1N, W2N = meta["W1N"], meta["W2N"]
    D1N, D2N, D3N = meta["D1N"], meta["D2N"], meta["D3N"]
    T1t, T2t, T3t = meta["T1t"], meta["T2t"], meta["T3t"]
    SW, PCW = meta["SW"], meta["PCW"]
    sh_offs, pc_offs = meta["sh_offs"], meta["pc_offs"]
    PCOL = 257
    CCW = W1N * PCOL

    nc = bacc.Bacc(None, target_bir_lowering=False, debug=True)
    pcb_d = nc.dram_tensor("pcb", [1, PCW], I16, kind="ExternalInput")
    shb_d = nc.dram_tensor("shb", [1, SW], I16, kind="ExternalInput")
    out_d = nc.dram_tensor("out", [B, H], F32, kind="ExternalOutput")

    ccin = nc.dram_tensor("ccin", [128, CCW], F32)
    ccout = nc.dram_tensor("ccout", [128, CCW], F32, addr_space="Shared")

    def _view(handle, offs, name, dtype, pattern, **axes):
        off, u = offs[name]
        ap = handle[0, off:off + u]
        if dtype != I16:
            ap = ap.bitcast(dtype)
        return ap.rearrange(pattern, **axes)

    def shv(name, dtype, pattern, **axes):
        return _view(ccw, sh_offs, name, dtype, pattern, **axes)

    def pcv(name, dtype, pattern, **axes):
        return _view(pcb_d, pc_offs, name, dtype, pattern, **axes)

    with tile.TileContext(nc, num_cores=NCORES) as tc:
        with tc.tile_pool(name="cst", bufs=1) as cst, \
             tc.tile_pool(name="drm", bufs=1, space="DRAM") as drm:
            t0hbm = drm.tile([SLICE, 384], BF16, name="t0hbm")
            t1hbm = drm.tile([D1N, 384], BF16, name="t1hbm")
            t2hbm = drm.tile([D2N, 384], BF16, name="t2hbm")
            ccw_in = drm.tile([1, SW], I16, name="ccw_in")
            ccw = drm.tile([1, SW * NCORES], I16, name="ccw")
            ccad_in = drm.tile([1, SLICE], F32, name="ccad_in")
            ccad = drm.tile([1, NTOT], F32, name="ccad")

            # ---- shared-blob AllGather (weights etc. transfer once, not 8x)
            nc.gpsimd.dma_start(out=ccw_in[:], in_=shb_d[:])
            nc.gpsimd.collective_compute(
                "AllGather", Alu.bypass, replica_groups=[list(range(NCORES))],
                ins=[ccw_in[:].opt()], outs=[ccw[:].opt()])

            def wload(out_ap, in_ap):
                return nc.sync.dma_start(out=out_ap, in_=in_ap)

            t_wconv = cst.tile([128, 3, 2, H], BF16)
            wload(t_wconv[:], shv("wconv", BF16, "(p a b m) -> p a b m", p=128, a=3, b=2))
            t_wext = cst.tile([128, 2, L, 258], BF16)
            wload(t_wext[:], shv("wext", BF16, "(p a b m) -> p a b m", p=128, a=2, b=L))
            t_wadc = cst.tile([128, 2, 1], BF16)
            wload(t_wadc[:], shv("wadc", BF16, "(p a o) -> p a o", p=128, a=2))
            t_bcol = cst.tile([128, L, 2, 1], F32)
            wload(t_bcol[:], shv("bcol", F32, "(p a b o) -> p a b o", p=128, a=L, b=2))
            t_tcb = cst.tile([128, 2, 1], F32)
            wload(t_tcb[:], shv("tcb", F32, "(p a o) -> p a o", p=128, a=2))
            t_c128 = cst.tile([128, 128], BF16)
            wload(t_c128[:], shv("c128", BF16, "(p n) -> p n", p=128))
            t_idf = cst.tile([128, 128], F32)
            wload(t_idf[:], shv("identf", F32, "(p n) -> p n", p=128))
            t_lng = cst.tile([B, H], F32)
            wload(t_lng[:], shv("lng", F32, "(b h) -> b h", b=B))
            t_lnb = cst.tile([B, H], F32)
            wload(t_lnb[:], shv("lnb", F32, "(b h) -> b h", b=B))
            t_d1c = cst.tile([128, W1N], BF16)
            wload(t_d1c[:], shv("d1col", BF16, "(p w) -> p w", p=128))

            def idx_load(name, cols):
                t = cst.tile([128, cols], I16, name=f"t_{name}")
                wload(t[0:16, :], shv(name, I16, "(p n) -> p n", p=16))
                nc.sync.dma_start(out=t[16:32, :], in_=t[0:16, :])
                nc.sync.dma_start(out=t[32:64, :], in_=t[0:32, :])
                nc.sync.dma_start(out=t[64:128, :], in_=t[0:64, :])
                return t

            t_g2i = idx_load("g2i", T2t * 8)
            t_g3i = idx_load("g3i", T3t * 8)
            t_d2i = idx_load("d2i", D2N // 16)
            t_d3i = idx_load("d3i", D3N // 16)
            t_d1i = idx_load("d1i", D1N // 16)
            t_g2s = cst.tile([128, T2t], BF16)
            wload(t_g2s[:], shv("g2s", BF16, "(p n) -> p n", p=128))
            t_g3s = cst.tile([128, T3t], BF16)
            wload(t_g3s[:], shv("g3s", BF16, "(p n) -> p n", p=128))

            # ---- per-core blob loads (no cc dependency)
            t_xtq = cst.tile([128, 2, SLICE + 2], FP8E4)
            nc.sync.dma_start(
                out=t_xtq[:],
                in_=pcv("xtq", FP8E4, "(kc p n) -> p kc n", p=128, n=SLICE + 2))
            t_xt = cst.tile([128, 2, SLICE + 2], BF16)
            nc.vector.tensor_copy(out=t_xt[:], in_=t_xtq[:])

            t_g1i = cst.tile([128, T1t * 8], I16)
            nc.sync.dma_start(out=t_g1i[0:16, :],
                              in_=pcv("g1i", I16, "(p n) -> p n", p=16))
            nc.sync.dma_start(out=t_g1i[16:32, :], in_=t_g1i[0:16, :])
            nc.sync.dma_start(out=t_g1i[32:64, :], in_=t_g1i[0:32, :])
            nc.sync.dma_start(out=t_g1i[64:128, :], in_=t_g1i[0:64, :])
            t_g1s = cst.tile([128, T1t], BF16)
            nc.sync.dma_start(out=t_g1s[:], in_=pcv("g1s", BF16, "(p n) -> p n", p=128))

            t_ones = cst.tile([1, 128], F32)
            nc.vector.memset(t_ones[:], 1.0)

            t_h0 = cst.tile([128, 2, SLICE], BF16)        # conv out, [ch, node]
            t_ad1 = cst.tile([1, D1N], F32)               # ad row, D1-compact
            t_part = cst.tile([128, CCW], F32)            # partials (pre-AllReduce)
            t_psum = cst.tile([128, CCW], F32)            # partials (post-AllReduce)

            # =========================== stage A: conv on slice
            with tc.tile_pool(name="psA", bufs=2, space="PSUM") as psA, \
                 tc.tile_pool(name="wkA", bufs=5) as wkA:
                NCH = SLICE // 512
                for mb in range(2):
                    for nchunk in range(NCH):
                        ps = psA.tile([128, 512], F32, tag="cv")
                        first = True
                        for k in range(3):
                            for kc in range(2):
                                nc.tensor.matmul(
                                    ps[:],
                                    lhsT=t_wconv[:, k, kc, mb * 128:(mb + 1) * 128],
                                    rhs=t_xt[:, kc, nchunk * 512 + k: nchunk * 512 + k + 512],
                                    start=first, stop=(k == 2 and kc == 1),
                                )
                                first = False
                        tmp = wkA.tile([128, 512], F32, tag="cvt")
                        nc.vector.tensor_tensor(
                            out=tmp[:], in0=ps[:],
                            in1=t_xt[:, mb, nchunk * 512 + 1: nchunk * 512 + 513],
                            op=Alu.add)
                        nc.vector.tensor_scalar(
                            out=t_h0[:, mb, nchunk * 512:(nchunk + 1) * 512],
                            in0=tmp[:], scalar1=t_tcb[:, mb, :], scalar2=0.0,
                            op0=Alu.add, op1=Alu.max)

                # ============ stage B': layer-1 ad row via tiny AllGather
                t_adrow = wkA.tile([1, SLICE], F32, tag="adrow")
                for j in range(4):
                    ps = psA.tile([1, 512], F32, tag="ad")
                    for kc in range(2):
                        nc.tensor.matmul(
                            ps[:], lhsT=t_wadc[:, kc, :],
                            rhs=t_h0[:, kc, j * 512:(j + 1) * 512],
                            start=(kc == 0), stop=(kc == 1))
                    nc.vector.tensor_copy(out=t_adrow[:, j * 512:(j + 1) * 512], in_=ps[:])
                nc.gpsimd.dma_start(out=ccad_in[:], in_=t_adrow[:])
                nc.gpsimd.collective_compute(
                    "AllGather", Alu.bypass, replica_groups=[list(range(NCORES))],
                    ins=[ccad_in[:].opt()], outs=[ccad[:].opt()])
                Gad = wkA.tile([128, W1N, 64], F32, tag="Gad")
                for ch0 in range(0, W1N, 4):  # <=512 idxs per SWDGE gather
                    chn = min(4, W1N - ch0)
                    nc.gpsimd.dma_gather(
                        out_ap=Gad[:, ch0:ch0 + chn, :],
                        in_ap=ccad[:].rearrange("o (r e) -> (o r) e", e=64),
                        idxs_ap=t_d1i[:, ch0 * 8:(ch0 + chn) * 8],
                        num_idxs=chn * 128, num_idxs_reg=chn * 128,
                        elem_size=64)
                t_sel = wkA.tile([128, W1N, 64], BF16, tag="sel")
                nc.vector.tensor_tensor(
                    out=t_sel[:],
                    in0=t_c128[:, 0:64].unsqueeze(1).broadcast_to([128, W1N, 64]),
                    in1=t_d1c[:].unsqueeze(2).broadcast_to([128, W1N, 64]),
                    op=Alu.is_equal)
                t_gsel = wkA.tile([128, W1N, 64], F32, tag="gsel")
                nc.vector.tensor_tensor(out=t_gsel[:], in0=Gad[:], in1=t_sel[:], op=Alu.mult)
                t_ad1x = wkA.tile([128, W1N], F32, tag="ad1x")
                nc.vector.tensor_reduce(
                    out=t_ad1x[:], in_=t_gsel[:], axis=mybir.AxisListType.X, op=Alu.add)
                ps_t = psA.tile([W1N, 128], F32, tag="adT")
                nc.tensor.matmul(ps_t[:], t_ad1x[:], t_idf[:], is_transpose=True)
                t_adr = wkA.tile([W1N, 128], F32, tag="adr")
                nc.vector.tensor_copy(out=t_adr[:], in_=ps_t[:])
                nc.sync.dma_start(
                    out=t_ad1[:].rearrange("o (w j) -> o w j", j=128), in_=t_adr[:])

                # ======================= stage C: transform0 -> T0 table
                for nb in range(SLICE // 128):
                    ps = psA.tile([128, 258], F32, tag="tr")
                    for kc in range(2):
                        nc.tensor.matmul(
                            ps[:], lhsT=t_h0[:, kc, nb * 128:(nb + 1) * 128],
                            rhs=t_wext[:, kc, 0, :], start=(kc == 0), stop=(kc == 1))
                    stg = wkA.tile([128, 384], BF16, tag="stg")
                    nc.scalar.copy(out=stg[:, 0:258], in_=ps[:, 0:258])
                    nc.vector.memset(stg[:, 262:384], 0.0)
                    nc.vector.tensor_copy(
                        out=stg[:].bitcast(F32)[:, 129:131], in_=ps[:, 256:258])
                    nc.sync.dma_start(
                        out=t0hbm[nb * 128:(nb + 1) * 128, :], in_=stg[:])

            # =========================== stage D: layer-1 partials
            def edge_window(wi, Tw, off, t_gi, t_gs, table, ad_src,
                            ps_pool, wk, tagp, Tmax):
                G = wk.tile([128, Tmax, 384], BF16, tag=tagp + "G")
                GMAX = 4
                for t0 in range(0, Tw, GMAX):
                    tn = min(GMAX, Tw - t0)
                    nc.gpsimd.dma_gather(
                        out_ap=G[:, t0: t0 + tn, :], in_ap=table[:, :],
                        idxs_ap=t_gi[:, (off + t0) * 8: (off + t0 + tn) * 8],
                        num_idxs=tn * 128, num_idxs_reg=tn * 128, elem_size=384)
                M = wk.tile([128, Tmax, 128], BF16, tag=tagp + "M")
                nc.vector.tensor_tensor(
                    out=M[:, :Tw, :],
                    in0=t_c128[:].unsqueeze(1).broadcast_to([128, Tw, 128]),
                    in1=t_gs[:, off: off + Tw].unsqueeze(2).broadcast_to([128, Tw, 128]),
                    op=Alu.is_equal)
                ps1, ps2 = ps_pool
                ps_ad = ps1.tile([128, 128], F32, tag="rep")
                nc.tensor.matmul(ps_ad[:], lhsT=t_ones[:], rhs=ad_src, start=True, stop=True)
                tmp = wk.tile([128, Tmax, 128], F32, tag=tagp + "tmp")
                nc.vector.tensor_tensor(
                    out=tmp[:, :Tw, :], in0=M[:, :Tw, :],
                    in1=ps_ad[:].unsqueeze(1).broadcast_to([128, Tw, 128]), op=Alu.mult)
                adx = wk.tile([128, Tmax], F32, tag=tagp + "adx")
                nc.vector.tensor_reduce(
                    out=adx[:, :Tw], in_=tmp[:, :Tw, :], axis=mybir.AxisListType.X, op=Alu.add)
                lg = wk.tile([128, Tmax], F32, tag=tagp + "lg")
                nc.vector.tensor_tensor(
                    out=lg[:, :Tw], in0=G[:, :Tw, :].bitcast(F32)[:, :, 129],
                    in1=adx[:, :Tw], op=Alu.add)
                l2 = wk.tile([128, Tmax], F32, tag=tagp + "l2")
                nc.vector.tensor_scalar(
                    out=l2[:, :Tw], in0=lg[:, :Tw], scalar1=NEG, scalar2=None, op0=Alu.mult)
                nc.vector.tensor_tensor(out=lg[:, :Tw], in0=lg[:, :Tw], in1=l2[:, :Tw], op=Alu.max)
                ex = wk.tile([128, Tmax], F32, tag=tagp + "ex")
                nc.scalar.activation(out=ex[:, :Tw], in_=lg[:, :Tw], func=Act.Exp)
                exb = wk.tile([128, Tmax], BF16, tag=tagp + "exb")
                nc.vector.tensor_copy(out=exb[:, :Tw], in_=ex[:, :Tw])
                Mex = wk.tile([128, Tmax, 128], BF16, tag=tagp + "Mex")
                nc.vector.tensor_tensor(
                    out=Mex[:, :Tw, :], in0=M[:, :Tw, :],
                    in1=exb[:, :Tw].unsqueeze(2).broadcast_to([128, Tw, 128]), op=Alu.mult)
                ps_a = ps2.tile([128, 128], F32, tag="agg")
                ps_b = ps2.tile([128, 128], F32, tag="agg")
                ps_d = ps1.tile([128, 1], F32, tag="den")
                for t in range(Tw):
                    nc.tensor.matmul(ps_a[:], lhsT=G[:, t, 0:128], rhs=Mex[:, t, :],
                                     start=(t == 0), stop=(t == Tw - 1))
                    nc.tensor.matmul(ps_b[:], lhsT=G[:, t, 128:256], rhs=Mex[:, t, :],
                                     start=(t == 0), stop=(t == Tw - 1))
                    nc.tensor.matmul(ps_d[:], lhsT=M[:, t, :], rhs=exb[:, t: t + 1],
                                     start=(t == 0), stop=(t == Tw - 1))
                return ps_a, ps_b, ps_d

            T1max = max(T1w)
            with tc.tile_pool(name="psD", bufs=2, space="PSUM") as psD, \
                 tc.tile_pool(name="psD2", bufs=4, space="PSUM") as psD2, \
                 tc.tile_pool(name="wkD", bufs=4) as wkD:
                off = 0
                for wi in range(W1N):
                    Tw = T1w[wi]
                    pa, pb, pd = edge_window(
                        wi, Tw, off, t_g1i, t_g1s, t0hbm,
                        t_ad1[:, wi * 128:(wi + 1) * 128],
                        (psD, psD2), wkD, "w1", T1max)
                    nc.scalar.copy(out=t_part[:, wi * PCOL: wi * PCOL + 128], in_=pa[:])
                    nc.scalar.copy(out=t_part[:, wi * PCOL + 128: wi * PCOL + 256], in_=pb[:])
                    nc.vector.tensor_copy(out=t_part[:, wi * PCOL + 256: wi * PCOL + 257], in_=pd[:])
                    off += Tw

            # =========================== stage E: AllReduce partials
            d_in = nc.gpsimd.dma_start(out=ccin[:], in_=t_part[:])
            cc = nc.gpsimd.collective_compute(
                "AllReduce", Alu.add, replica_groups=[list(range(NCORES))],
                ins=[ccin[:]], outs=[ccout[:]])
            adep(cc.ins, d_in.ins, sync=True, reason="cc after partials write")
            d_out = nc.gpsimd.dma_start(out=t_psum[:], in_=ccout[:])
            adep(d_out.ins, cc.ins, sync=True, reason="readback after cc")

            # =========================== stage F: finalize h1 + T1
            with tc.tile_pool(name="psF", bufs=2, space="PSUM") as psF, \
                 tc.tile_pool(name="wkF", bufs=2) as wkF:
                denc = wkF.tile([128, W1N], F32, tag="denc")
                nc.vector.tensor_copy(
                    out=denc[:],
                    in_=t_psum[:].rearrange("p (w q) -> p w q", q=PCOL)[:, :, 256])
                ps_rows = psF.tile([W1N, 128], F32, tag="rows")
                nc.tensor.matmul(ps_rows[:], denc[:], t_idf[:], is_transpose=True)
                recr = wkF.tile([W1N, 128], F32, tag="recr")
                nc.vector.tensor_scalar(
                    out=recr[:], in0=ps_rows[:], scalar1=1e-20, scalar2=None, op0=Alu.add)
                nc.vector.reciprocal(out=recr[:], in_=recr[:])
                recf = wkF.tile([1, W1N * 128], F32, tag="recf")
                nc.sync.dma_start(
                    out=recf[:].rearrange("o (w j) -> o w j", j=128), in_=recr[:])

                for wi in range(W1N):
                    ps_r = psF.tile([128, 128], F32, tag="rep")
                    nc.tensor.matmul(ps_r[:], lhsT=t_ones[:],
                                     rhs=recf[:, wi * 128:(wi + 1) * 128],
                                     start=True, stop=True)
                    h1 = wkF.tile([128, 2, 128], BF16, tag="h1")
                    for mb in range(2):
                        sc = wkF.tile([128, 128], F32, tag="sc")
                        nc.vector.tensor_tensor(
                            out=sc[:], in0=t_psum[:, wi * PCOL + mb * 128: wi * PCOL + (mb + 1) * 128],
                            in1=ps_r[:], op=Alu.mult)
                        nc.vector.tensor_scalar(
                            out=h1[:, mb, :], in0=sc[:],
                            scalar1=t_bcol[:, 0, mb, :], scalar2=0.0,
                            op0=Alu.add, op1=Alu.max)
                    ps_t = psF.tile([128, 258], F32, tag="tr")
                    for kc in range(2):
                        nc.tensor.matmul(ps_t[:], lhsT=h1[:, kc, :], rhs=t_wext[:, kc, 1, :],
                                         start=(kc == 0), stop=(kc == 1))
                    stg = wkF.tile([128, 384], BF16, tag="stg1")
                    nc.scalar.copy(out=stg[:, 0:258], in_=ps_t[:, 0:258])
                    nc.vector.memset(stg[:, 262:384], 0.0)
                    nc.vector.tensor_copy(
                        out=stg[:].bitcast(F32)[:, 129:131], in_=ps_t[:, 256:258])
                    nc.sync.dma_start(out=t1hbm[wi * 128:(wi + 1) * 128, :], in_=stg[:])

            # =========================== stages G/H: layers 2 and 3 (redundant)
            def small_layer(lidx, WN, Twl, Ttot, t_gi, t_gs, t_didx, table,
                            out_table, psP, wkP, tagp, dntag):
                Tmax = max(Twl)
                Gd = wkP.tile([128, WN, 384], BF16, tag=tagp + "Gd")
                nc.gpsimd.dma_gather(
                    out_ap=Gd[:, :, :], in_ap=table[:, :], idxs_ap=t_didx[:, :],
                    num_idxs=WN * 128, num_idxs_reg=WN * 128, elem_size=384)
                adc = wkP.tile([128, WN], F32, tag=tagp + "adc")
                nc.vector.tensor_copy(out=adc[:], in_=Gd[:, :, :].bitcast(F32)[:, :, 130])
                ps1, _ = psP
                ps_rows = ps1.tile([WN, 128], F32, tag=dntag)
                nc.tensor.matmul(ps_rows[:], adc[:], t_idf[:], is_transpose=True)
                adrows = wkP.tile([WN, 128], F32, tag=tagp + "adr")
                nc.vector.tensor_copy(out=adrows[:], in_=ps_rows[:])
                adf = wkP.tile([1, WN * 128], F32, tag=tagp + "adf")
                nc.sync.dma_start(
                    out=adf[:].rearrange("o (w j) -> o w j", j=128), in_=adrows[:])

                part2 = wkP.tile([128, WN, 257], F32, tag=tagp + "pt")
                denc = wkP.tile([128, WN], F32, tag=tagp + "dc")
                off = 0
                for wi in range(WN):
                    Tw = Twl[wi]
                    pa, pb, pd = edge_window(
                        wi, Tw, off, t_gi, t_gs, table,
                        adf[:, wi * 128:(wi + 1) * 128], psP, wkP, tagp, Tmax)
                    nc.scalar.copy(out=part2[:, wi, 0:128], in_=pa[:])
                    nc.scalar.copy(out=part2[:, wi, 128:256], in_=pb[:])
                    nc.vector.tensor_copy(out=denc[:, wi: wi + 1], in_=pd[:])
                    off += Tw
                ps1, _ = psP
                ps_dr = ps1.tile([WN, 128], F32, tag=dntag)
                nc.tensor.matmul(ps_dr[:], denc[:], t_idf[:], is_transpose=True)
                recr = wkP.tile([WN, 128], F32, tag=tagp + "rc")
                nc.vector.tensor_scalar(out=recr[:], in0=ps_dr[:], scalar1=1e-20,
                                        scalar2=None, op0=Alu.add)
                nc.vector.reciprocal(out=recr[:], in_=recr[:])
                rcf = wkP.tile([1, WN * 128], F32, tag=tagp + "rcf")
                nc.sync.dma_start(
                    out=rcf[:].rearrange("o (w j) -> o w j", j=128), in_=recr[:])
                hts = []
                for wi in range(WN):
                    ps_r = ps1.tile([128, 128], F32, tag="rep")
                    nc.tensor.matmul(ps_r[:], lhsT=t_ones[:],
                                     rhs=rcf[:, wi * 128:(wi + 1) * 128],
                                     start=True, stop=True)
                    ht = wkP.tile([128, 2, 128], BF16, tag=tagp + "ht")
                    for mb in range(2):
                        sc = wkP.tile([128, 128], F32, tag=tagp + "sc")
                        nc.vector.tensor_tensor(out=sc[:], in0=part2[:, wi, mb * 128:(mb + 1) * 128],
                                                in1=ps_r[:], op=Alu.mult)
                        nc.vector.tensor_scalar(
                            out=ht[:, mb, :], in0=sc[:],
                            scalar1=t_bcol[:, lidx, mb, :], scalar2=0.0,
                            op0=Alu.add, op1=Alu.max)
                    hts.append(ht)
                    if out_table is not None:
                        ps_t = ps1.tile([128, 258], F32, tag="tr")
                        for kc in range(2):
                            nc.tensor.matmul(ps_t[:], lhsT=ht[:, kc, :],
                                             rhs=t_wext[:, kc, lidx + 1, :],
                                             start=(kc == 0), stop=(kc == 1))
                        stg = wkP.tile([128, 384], BF16, tag=tagp + "st")
                        nc.scalar.copy(out=stg[:, 0:258], in_=ps_t[:, 0:258])
                        nc.vector.memset(stg[:, 262:384], 0.0)
                        nc.vector.tensor_copy(
                            out=stg[:].bitcast(F32)[:, 129:131], in_=ps_t[:, 256:258])
                        nc.sync.dma_start(
                            out=out_table[wi * 128:(wi + 1) * 128, :], in_=stg[:])
                return None, hts

            with tc.tile_pool(name="psG", bufs=1, space="PSUM") as psG, \
                 tc.tile_pool(name="psG2", bufs=2, space="PSUM") as psG2, \
                 tc.tile_pool(name="wkG", bufs=2) as wkG:
                _ = small_layer(
                    1, W2N, T2w, T2t, t_g2i, t_g2s, t_d2i, t1hbm,
                    t2hbm, (psG, psG2), wkG, "w2", "dn")
                _, h3ts = small_layer(
                    2, 1, T3w, T3t, t_g3i, t_g3s, t_d3i, t2hbm,
                    None, (psG, psG2), wkG, "w3", "dn")
                h3 = h3ts[0]

                # ===================== stage I: final conv (position 0 only)
                fin = []
                for mb in range(2):
                    ps = psG.tile([128, B], F32, tag="fc")
                    first = True
                    for kc in range(2):
                        for k in (1, 2):
                            nc.tensor.matmul(
                                ps[:], lhsT=t_wconv[:, k, kc, mb * 128:(mb + 1) * 128],
                                rhs=h3[:, kc, :].rearrange("p (g two) -> p two g", two=2)[:, k - 1, 0:B],
                                start=first, stop=(kc == 1 and k == 2))
                            first = False
                    ot = wkG.tile([128, B], F32, tag="fo")
                    nc.vector.tensor_scalar(out=ot[:], in0=ps[:], scalar1=t_tcb[:, mb, :],
                                            scalar2=None, op0=Alu.add)
                    fin.append(ot)

                # ===================== stage J: layernorm over channels + relu
                o4 = wkG.tile([B, 2, 128], F32, tag="o4")
                for mb in range(2):
                    ps = psG.tile([B, 128], F32, tag="tp4")
                    nc.tensor.matmul(ps[:], fin[mb][:], t_idf[:], is_transpose=True)
                    nc.vector.tensor_copy(out=o4[:, mb, :], in_=ps[:])
                ov = o4[:].rearrange("b m j -> b (m j)")
                mean = wkG.tile([B, 1], F32, tag="mn")
                nc.vector.tensor_reduce(out=mean[:], in_=ov, axis=mybir.AxisListType.X,
                                        op=Alu.add, negate=True)
                nc.vector.tensor_scalar(out=mean[:], in0=mean[:], scalar1=1.0 / H,
                                        scalar2=None, op0=Alu.mult)
                xc = wkG.tile([B, H], F32, tag="xc")
                nc.vector.tensor_scalar(out=xc[:], in0=ov, scalar1=mean[:],
                                        scalar2=None, op0=Alu.add)
                sq = wkG.tile([B, H], F32, tag="sq")
                nc.vector.tensor_tensor(out=sq[:], in0=xc[:], in1=xc[:], op=Alu.mult)
                var = wkG.tile([B, 1], F32, tag="vr")
                nc.vector.tensor_reduce(out=var[:], in_=sq[:], axis=mybir.AxisListType.X,
                                        op=Alu.add)
                nc.vector.tensor_scalar(out=var[:], in0=var[:], scalar1=1.0 / H,
                                        scalar2=EPS, op0=Alu.mult, op1=Alu.add)
                nc.scalar.activation(out=var[:], in_=var[:], func=Act.Sqrt)
                nc.vector.reciprocal(out=var[:], in_=var[:])
                res = wkG.tile([B, H], F32, tag="res")
                nc.vector.tensor_scalar(out=res[:], in0=xc[:], scalar1=var[:],
                                        scalar2=None, op0=Alu.mult)
                nc.vector.tensor_tensor(out=res[:], in0=res[:], in1=t_lng[:], op=Alu.mult)
                nc.vector.tensor_tensor(out=res[:], in0=res[:], in1=t_lnb[:], op=Alu.add)
                nc.vector.tensor_scalar(out=res[:], in0=res[:], scalar1=0.0,
                                        scalar2=None, op0=Alu.max)
                nc.sync.dma_start(out=out_d[:], in_=res[:])

    nc.compile()
    return nc


# ---------------------------------------------------------------- entry point
def kernel(**inputs):
    from concourse.bass_utils import run_bass_kernel_spmd

    in_maps, meta = _prep(**inputs)
    nc = _build(meta)
    res = run_bass_kernel_spmd(nc, in_maps, list(range(NCORES)))
    return np.asarray(res.results[0]["out"], np.float32)
